# revision 5
# baseline (speedup 1.0000x reference)
"""Trainium2 Bass kernel for nn_ConditionalModuleBGR (histogram binning + tiny MLP).

Strategy: data-parallel over 8 NeuronCores, 2 images per core. Per core:
  - 6 slabs (2 images x 3 channels) of 1M pixels each, viewed as [128, 8192] f32.
  - Per slab, count pixels >= b/64 for b=1..63 via fused
    tensor_scalar(is_ge, reduce-add) on the Vector engine, plus a parallel
    channel on the Scalar engine using hard-saturated sigmoid + accumulate.
  - hist[b] = C[b] - C[b+1] (adjacent differences of the >= counts), then a
    ones-matmul on the Tensor engine reduces across partitions and lands the
    features pre-transposed for the MLP.
  - Tiny MLP (192->128 relu ->32) on the Tensor engine, sigmoid on Scalar.
Output per core: [2, 32]; host concatenates to [16, 32].
"""

import numpy as np

import concourse.bacc as bacc
import concourse.mybir as mybir
import concourse.tile as tile
from concourse import bass_utils

N_CORES = 8
N_IMG = 16
IMG_PER_CORE = N_IMG // N_CORES  # 2
CH = 3
SLABS = IMG_PER_CORE * CH  # 6
P = 128
F = 8192  # 1024*1024 / 128
BINS = 64
FEAT = CH * BINS  # 192
HID = 128
OUT = 32
W1_N = FEAT * HID  # 24576
B1_N = HID
W2_N = HID * OUT  # 4096
B2_N = OUT
G_OFF = W1_N + B1_N + W2_N + B2_N  # 28832
N_PARAMS = 28864

# Threshold split between engines (b in 1..63). ACT (scalar engine) takes
# every act_frac-th threshold; tuned from measured pass rates.
N_ACT = 24

F32 = mybir.dt.float32


def _act_thresholds():
    """Pick N_ACT threshold indices (1..63) for the scalar engine, evenly."""
    if N_ACT == 0:
        return set()
    idx = np.linspace(1, 63, N_ACT).round().astype(int)
    return set(int(i) for i in idx)


def _build():
    nc = bacc.Bacc("TRN2", target_bir_lowering=False, debug=False,
                   num_devices=N_CORES)
    img = nc.dram_tensor("img", [SLABS, P, F], F32, kind="ExternalInput")
    params = nc.dram_tensor("params", [N_PARAMS], F32, kind="ExternalInput")
    out = nc.dram_tensor("out", [IMG_PER_CORE, OUT], F32, kind="ExternalOutput")

    img_ap = img.ap()
    par_ap = params.ap()
    out_ap = out.ap()

    act_set = _act_thresholds()

    with tile.TileContext(nc) as tc:
        with (
            tc.tile_pool(name="data", bufs=2) as data_pool,
            tc.tile_pool(name="work", bufs=1) as work,
            tc.tile_pool(name="psum", bufs=1, space="PSUM") as psum,
        ):
            scratch = work.tile([P, F], F32, tag="scratch")
            scratch2 = work.tile([P, F], F32, tag="scratch2")
            cnt = work.tile([P, SLABS * BINS], F32, tag="cnt")
            dcnt = work.tile([P, SLABS * BINS], F32, tag="dcnt")
            ones = work.tile([P, 1], F32, tag="ones")
            nc.vector.memset(ones[:], 1.0)
            act_biases = work.tile([P, BINS], F32, tag="act_biases")
            for b in sorted(act_set):
                nc.gpsimd.memset(act_biases[:, b:b + 1], -float(b) * (2.0 ** 32))
            # C_0 (count >= 0) is all of each partition's elements.
            nc.vector.memset(cnt[:, 0:SLABS * BINS:BINS], float(F))

            # --- threshold counting ---
            for s in range(SLABS):
                xt = data_pool.tile([P, F], F32, tag="xt")
                nc.sync.dma_start(out=xt[:], in_=img_ap[s])
                for b in range(1, BINS):
                    col = cnt[:, s * BINS + b : s * BINS + b + 1]
                    t = float(b) / BINS
                    if b in act_set:
                        # count via hard-saturated sigmoid: exact except for
                        # x == t ties (each contributes 0.5; negligible).
                        nc.scalar.activation(
                            out=scratch2[:], in_=xt[:],
                            func=mybir.ActivationFunctionType.Sigmoid,
                            scale=float(2.0 ** 38),
                            bias=act_biases[:, b:b + 1],
                            accum_out=col,
                        )
                    else:
                        nc.vector.tensor_scalar(
                            out=scratch[:], in0=xt[:],
                            scalar1=t, scalar2=0.0,
                            op0=mybir.AluOpType.is_ge,
                            op1=mybir.AluOpType.add,
                            accum_out=col,
                        )

            # --- hist[b] = C[b] - C[b+1]; per-partition, then reduce via PE ---
            nbm1 = SLABS * BINS - 1
            nc.vector.tensor_tensor(
                out=dcnt[:, 0:nbm1], in0=cnt[:, 0:nbm1], in1=cnt[:, 1:nbm1 + 1],
                op=mybir.AluOpType.subtract,
            )
            # last bin of each slab: hist[63] = C[63] (count >= 1.0 is 0)
            nc.vector.tensor_copy(
                out=dcnt[:, BINS - 1:SLABS * BINS:BINS],
                in_=cnt[:, BINS - 1:SLABS * BINS:BINS],
            )

            # featT[f, i] = hist of image i, feature f = c*64+b.
            # dcnt columns for image i are contiguous: [i*192, (i+1)*192).
            featT_a = psum.tile([P, IMG_PER_CORE], F32, tag="featTa")
            featT_b = psum.tile([BINS, IMG_PER_CORE], F32, tag="featTb")
            for i in range(IMG_PER_CORE):
                base = i * FEAT
                nc.tensor.matmul(
                    featT_a[:, i:i + 1], dcnt[:, base:base + P], ones[:],
                    start=True, stop=True,
                )
                nc.tensor.matmul(
                    featT_b[:, i:i + 1], dcnt[:, base + P:base + FEAT], ones[:],
                    start=True, stop=True,
                )

            feat_a = work.tile([P, IMG_PER_CORE], F32, tag="feata")
            feat_b = work.tile([BINS, IMG_PER_CORE], F32, tag="featb")
            nc.vector.tensor_copy(out=feat_a[:], in_=featT_a[:])
            nc.vector.tensor_copy(out=feat_b[:], in_=featT_b[:])

            # --- MLP weights from params ---
            w1a = work.tile([P, HID], F32, tag="w1a")
            w1b = work.tile([FEAT - P, HID], F32, tag="w1b")
            nc.sync.dma_start(
                out=w1a[:], in_=par_ap[0:P * HID].rearrange("(a b) -> a b", a=P))
            nc.sync.dma_start(
                out=w1b[:],
                in_=par_ap[P * HID:W1_N].rearrange("(a b) -> a b", a=FEAT - P))
            b1 = work.tile([HID, 1], F32, tag="b1")
            nc.sync.dma_start(
                out=b1[:], in_=par_ap[W1_N:W1_N + B1_N].rearrange("(a b) -> a b", a=HID))
            w2 = work.tile([HID, OUT], F32, tag="w2")
            nc.sync.dma_start(
                out=w2[:],
                in_=par_ap[W1_N + B1_N:W1_N + B1_N + W2_N].rearrange(
                    "(a b) -> a b", a=HID))
            b2 = work.tile([OUT, 1], F32, tag="b2")
            nc.sync.dma_start(
                out=b2[:],
                in_=par_ap[W1_N + B1_N + W2_N:G_OFF].rearrange("(a b) -> a b", a=OUT))
            gsc = work.tile([1, 1], F32, tag="gsc")
            nc.sync.dma_start(
                out=gsc[:], in_=par_ap[G_OFF:G_OFF + 1].rearrange("(a b) -> a b", a=1))
            ones_out = work.tile([1, OUT], F32, tag="ones_out")
            nc.vector.memset(ones_out[:], 1.0)

            # broadcast global scalar to 32 partitions via PE
            g_psum = psum.tile([OUT, 1], F32, tag="gpsum")
            nc.tensor.matmul(g_psum[:], ones_out[:], gsc[:], start=True, stop=True)
            bias2 = work.tile([OUT, 1], F32, tag="bias2")
            nc.vector.tensor_add(out=bias2[:], in0=b2[:], in1=g_psum[:])

            # --- layer 1: h = relu(featT.T @ w1 + b1), computed transposed ---
            h_psum = psum.tile([HID, IMG_PER_CORE], F32, tag="hpsum")
            nc.tensor.matmul(h_psum[:], w1a[:], feat_a[:], start=True, stop=False)
            nc.tensor.matmul(h_psum[:], w1b[:], feat_b[:], start=False, stop=True)
            h = work.tile([HID, IMG_PER_CORE], F32, tag="h")
            # h = relu(h_psum + b1): scalar engine, bias is per-partition AP
            nc.scalar.activation(
                out=h[:], in_=h_psum[:],
                func=mybir.ActivationFunctionType.Relu,
                bias=b1[:], scale=1.0,
            )

            # --- layer 2: o = sigmoid(h.T @ w2 + b2 + g), transposed ---
            o_psum = psum.tile([OUT, IMG_PER_CORE], F32, tag="opsum")
            nc.tensor.matmul(o_psum[:], w2[:], h[:], start=True, stop=True)
            o = work.tile([OUT, IMG_PER_CORE], F32, tag="o")
            nc.scalar.activation(
                out=o[:], in_=o_psum[:],
                func=mybir.ActivationFunctionType.Sigmoid,
                bias=bias2[:], scale=1.0,
            )

            # --- store transposed [OUT, IMG] -> dram [IMG, OUT] ---
            nc.sync.dma_start(
                out=out_ap.rearrange("a b -> b a"), in_=o[:])

    nc.compile()
    return nc


_NC_CACHE = {}


def _get_nc():
    if "nc" not in _NC_CACHE:
        _NC_CACHE["nc"] = _build()
    return _NC_CACHE["nc"]


def kernel(img: np.ndarray, params: np.ndarray) -> np.ndarray:
    img = np.ascontiguousarray(img, dtype=np.float32)
    params = np.ascontiguousarray(params, dtype=np.float32)
    assert img.shape == (N_IMG, CH, 1024, 1024)
    assert params.shape == (N_PARAMS,)

    nc = _get_nc()
    shards = img.reshape(N_CORES, SLABS, P, F)
    in_maps = [
        {"img": shards[c], "params": params} for c in range(N_CORES)
    ]
    res = bass_utils.run_bass_kernel_spmd(nc, in_maps, core_ids=list(range(N_CORES)))
    return np.concatenate([res.results[c]["out"] for c in range(N_CORES)], axis=0)


# revision 6
# speedup vs baseline: 1.3161x; 1.3161x over previous
"""Trainium2 Bass kernel for nn_ConditionalModuleBGR (histogram binning + tiny MLP).

Strategy: data-parallel over 8 NeuronCores, 2 images per core. Per core:
  - 6 slabs (2 images x 3 channels) of 1M pixels each, viewed as [128, 8192] f32.
  - Per slab, count pixels >= b/64 for b=1..63 via fused
    tensor_scalar(is_ge, reduce-add) on the Vector engine, plus a parallel
    channel on the Scalar engine using hard-saturated sigmoid + accumulate.
  - hist[b] = C[b] - C[b+1] (adjacent differences of the >= counts), then a
    ones-matmul on the Tensor engine reduces across partitions and lands the
    features pre-transposed for the MLP.
  - Tiny MLP (192->128 relu ->32) on the Tensor engine, sigmoid on Scalar.
Output per core: [2, 32]; host concatenates to [16, 32].
"""

import numpy as np

import concourse.bacc as bacc
import concourse.mybir as mybir
import concourse.tile as tile
from concourse import bass_utils

N_CORES = 8
N_IMG = 16
IMG_PER_CORE = N_IMG // N_CORES  # 2
CH = 3
SLABS = IMG_PER_CORE * CH  # 6
P = 128
F = 8192  # 1024*1024 / 128
BINS = 64
FEAT = CH * BINS  # 192
HID = 128
OUT = 32
W1_N = FEAT * HID  # 24576
B1_N = HID
W2_N = HID * OUT  # 4096
B2_N = OUT
G_OFF = W1_N + B1_N + W2_N + B2_N  # 28832
N_PARAMS = 28864

# Threshold split between engines (b in 1..63). ACT (scalar engine) takes
# every act_frac-th threshold; tuned from measured pass rates.
N_ACT = 35

F32 = mybir.dt.float32


def _act_thresholds():
    """Pick N_ACT threshold indices (1..63) for the scalar engine, evenly."""
    if N_ACT == 0:
        return set()
    idx = np.linspace(1, 63, N_ACT).round().astype(int)
    return set(int(i) for i in idx)


def _build():
    nc = bacc.Bacc("TRN2", target_bir_lowering=False, debug=False,
                   num_devices=N_CORES)
    img = nc.dram_tensor("img", [SLABS, P, F], F32, kind="ExternalInput")
    params = nc.dram_tensor("params", [N_PARAMS], F32, kind="ExternalInput")
    out = nc.dram_tensor("out", [IMG_PER_CORE, OUT], F32, kind="ExternalOutput")

    img_ap = img.ap()
    par_ap = params.ap()
    out_ap = out.ap()

    act_set = _act_thresholds()

    with tile.TileContext(nc) as tc:
        with (
            tc.tile_pool(name="data", bufs=2) as data_pool,
            tc.tile_pool(name="work", bufs=1) as work,
            tc.tile_pool(name="psum", bufs=1, space="PSUM") as psum,
        ):
            scratch = work.tile([P, F], F32, tag="scratch")
            scratch2 = work.tile([P, F], F32, tag="scratch2")
            cnt = work.tile([P, SLABS * BINS], F32, tag="cnt")
            dcnt = work.tile([P, SLABS * BINS], F32, tag="dcnt")
            ones = work.tile([P, 1], F32, tag="ones")
            nc.vector.memset(ones[:], 1.0)
            act_biases = work.tile([P, BINS], F32, tag="act_biases")
            for b in sorted(act_set):
                nc.gpsimd.memset(act_biases[:, b:b + 1], -float(b) * (2.0 ** 32))
            # C_0 (count >= 0) is all of each partition's elements.
            nc.vector.memset(cnt[:, 0:SLABS * BINS:BINS], float(F))

            # --- threshold counting ---
            for s in range(SLABS):
                xt = data_pool.tile([P, F], F32, tag="xt")
                nc.sync.dma_start(out=xt[:], in_=img_ap[s])
                for b in range(1, BINS):
                    col = cnt[:, s * BINS + b : s * BINS + b + 1]
                    t = float(b) / BINS
                    if b in act_set:
                        # count via hard-saturated sigmoid: exact except for
                        # x == t ties (each contributes 0.5; negligible).
                        nc.scalar.activation(
                            out=scratch2[:], in_=xt[:],
                            func=mybir.ActivationFunctionType.Sigmoid,
                            scale=float(2.0 ** 38),
                            bias=act_biases[:, b:b + 1],
                            accum_out=col,
                        )
                    else:
                        nc.vector.tensor_scalar(
                            out=scratch[:], in0=xt[:],
                            scalar1=t, scalar2=0.0,
                            op0=mybir.AluOpType.is_ge,
                            op1=mybir.AluOpType.add,
                            accum_out=col,
                        )

            # --- hist[b] = C[b] - C[b+1]; per-partition, then reduce via PE ---
            nbm1 = SLABS * BINS - 1
            nc.vector.tensor_tensor(
                out=dcnt[:, 0:nbm1], in0=cnt[:, 0:nbm1], in1=cnt[:, 1:nbm1 + 1],
                op=mybir.AluOpType.subtract,
            )
            # last bin of each slab: hist[63] = C[63] (count >= 1.0 is 0)
            nc.vector.tensor_copy(
                out=dcnt[:, BINS - 1:SLABS * BINS:BINS],
                in_=cnt[:, BINS - 1:SLABS * BINS:BINS],
            )

            # featT[f, i] = hist of image i, feature f = c*64+b.
            # dcnt columns for image i are contiguous: [i*192, (i+1)*192).
            featT_a = psum.tile([P, IMG_PER_CORE], F32, tag="featTa")
            featT_b = psum.tile([BINS, IMG_PER_CORE], F32, tag="featTb")
            for i in range(IMG_PER_CORE):
                base = i * FEAT
                nc.tensor.matmul(
                    featT_a[:, i:i + 1], dcnt[:, base:base + P], ones[:],
                    start=True, stop=True,
                )
                nc.tensor.matmul(
                    featT_b[:, i:i + 1], dcnt[:, base + P:base + FEAT], ones[:],
                    start=True, stop=True,
                )

            feat_a = work.tile([P, IMG_PER_CORE], F32, tag="feata")
            feat_b = work.tile([BINS, IMG_PER_CORE], F32, tag="featb")
            nc.vector.tensor_copy(out=feat_a[:], in_=featT_a[:])
            nc.vector.tensor_copy(out=feat_b[:], in_=featT_b[:])

            # --- MLP weights from params ---
            w1a = work.tile([P, HID], F32, tag="w1a")
            w1b = work.tile([FEAT - P, HID], F32, tag="w1b")
            nc.sync.dma_start(
                out=w1a[:], in_=par_ap[0:P * HID].rearrange("(a b) -> a b", a=P))
            nc.sync.dma_start(
                out=w1b[:],
                in_=par_ap[P * HID:W1_N].rearrange("(a b) -> a b", a=FEAT - P))
            b1 = work.tile([HID, 1], F32, tag="b1")
            nc.sync.dma_start(
                out=b1[:], in_=par_ap[W1_N:W1_N + B1_N].rearrange("(a b) -> a b", a=HID))
            w2 = work.tile([HID, OUT], F32, tag="w2")
            nc.sync.dma_start(
                out=w2[:],
                in_=par_ap[W1_N + B1_N:W1_N + B1_N + W2_N].rearrange(
                    "(a b) -> a b", a=HID))
            b2 = work.tile([OUT, 1], F32, tag="b2")
            nc.sync.dma_start(
                out=b2[:],
                in_=par_ap[W1_N + B1_N + W2_N:G_OFF].rearrange("(a b) -> a b", a=OUT))
            gsc = work.tile([1, 1], F32, tag="gsc")
            nc.sync.dma_start(
                out=gsc[:], in_=par_ap[G_OFF:G_OFF + 1].rearrange("(a b) -> a b", a=1))
            ones_out = work.tile([1, OUT], F32, tag="ones_out")
            nc.vector.memset(ones_out[:], 1.0)

            # broadcast global scalar to 32 partitions via PE
            g_psum = psum.tile([OUT, 1], F32, tag="gpsum")
            nc.tensor.matmul(g_psum[:], ones_out[:], gsc[:], start=True, stop=True)
            bias2 = work.tile([OUT, 1], F32, tag="bias2")
            nc.vector.tensor_add(out=bias2[:], in0=b2[:], in1=g_psum[:])

            # --- layer 1: h = relu(featT.T @ w1 + b1), computed transposed ---
            h_psum = psum.tile([HID, IMG_PER_CORE], F32, tag="hpsum")
            nc.tensor.matmul(h_psum[:], w1a[:], feat_a[:], start=True, stop=False)
            nc.tensor.matmul(h_psum[:], w1b[:], feat_b[:], start=False, stop=True)
            h = work.tile([HID, IMG_PER_CORE], F32, tag="h")
            # h = relu(h_psum + b1): scalar engine, bias is per-partition AP
            nc.scalar.activation(
                out=h[:], in_=h_psum[:],
                func=mybir.ActivationFunctionType.Relu,
                bias=b1[:], scale=1.0,
            )

            # --- layer 2: o = sigmoid(h.T @ w2 + b2 + g), transposed ---
            o_psum = psum.tile([OUT, IMG_PER_CORE], F32, tag="opsum")
            nc.tensor.matmul(o_psum[:], w2[:], h[:], start=True, stop=True)
            o = work.tile([OUT, IMG_PER_CORE], F32, tag="o")
            nc.scalar.activation(
                out=o[:], in_=o_psum[:],
                func=mybir.ActivationFunctionType.Sigmoid,
                bias=bias2[:], scale=1.0,
            )

            # --- store transposed [OUT, IMG] -> dram [IMG, OUT] ---
            nc.sync.dma_start(
                out=out_ap.rearrange("a b -> b a"), in_=o[:])

    nc.compile()
    return nc


_NC_CACHE = {}


def _get_nc():
    if "nc" not in _NC_CACHE:
        _NC_CACHE["nc"] = _build()
    return _NC_CACHE["nc"]


def kernel(img: np.ndarray, params: np.ndarray) -> np.ndarray:
    img = np.ascontiguousarray(img, dtype=np.float32)
    params = np.ascontiguousarray(params, dtype=np.float32)
    assert img.shape == (N_IMG, CH, 1024, 1024)
    assert params.shape == (N_PARAMS,)

    nc = _get_nc()
    shards = img.reshape(N_CORES, SLABS, P, F)
    in_maps = [
        {"img": shards[c], "params": params} for c in range(N_CORES)
    ]
    res = bass_utils.run_bass_kernel_spmd(nc, in_maps, core_ids=list(range(N_CORES)))
    return np.concatenate([res.results[c]["out"] for c in range(N_CORES)], axis=0)


# revision 7
# speedup vs baseline: 1.8248x; 1.3865x over previous
"""Trainium2 Bass kernel for nn_ConditionalModuleBGR (histogram binning + tiny MLP).

Strategy: data-parallel over 8 NeuronCores, 2 images per core. Per core:
  - 6 slabs (2 images x 3 channels) of 1M pixels each, viewed as [128, 8192] f32.
  - Cumulative counts C[b] = #{x >= b/64} for b=1..63, split across engines:
      * Scalar engine: thresholds 1..N_ACT via hard-saturated sigmoid with
        fused accumulate (exact except x == t ties, which contribute 0.5).
      * Vector engine: thresholds N_ACT+1..63 via a custom packed DVE op that
        counts two thresholds per pass into 12-bit fields of one f32
        accumulator (chunked so each field stays < 4096; decoded exactly with
        integer-fp32 arithmetic).
  - hist[b] = C[b] - C[b+1]; ones-matmul on the Tensor engine reduces across
    partitions, landing features pre-transposed for the MLP.
  - Tiny MLP (192 -> 128 relu -> 32) on the Tensor engine, sigmoid on Scalar.
Output per core: [2, 32]; host concatenates to [16, 32].
"""

from operator import add as _op_add

import numpy as np

import concourse.bacc as bacc
import concourse.dve_ops as dve_ops
import concourse.mybir as mybir
import concourse.tile as tile
from concourse import bass_utils
from concourse.dve_ops import DveOp
from concourse.dve_spec import C0, C1, C2, Spec, Src0, lower
from concourse.dve_uop import DveOpSpec

N_CORES = 8
N_IMG = 16
IMG_PER_CORE = N_IMG // N_CORES  # 2
CH = 3
SLABS = IMG_PER_CORE * CH  # 6
P = 128
F = 8192  # 1024*1024 / 128
BINS = 64
FEAT = CH * BINS  # 192
HID = 128
OUT = 32
W1_N = FEAT * HID  # 24576
B1_N = HID
W2_N = HID * OUT  # 4096
B2_N = OUT
G_OFF = W1_N + B1_N + W2_N + B2_N  # 28832
N_PARAMS = 28864

# Engine split: ACT takes thresholds 1..N_ACT; DVE takes the rest in pairs.
N_ACT = 25
N_DVE = 63 - N_ACT  # must be even
NPAIR = N_DVE // 2
PAIR_LO = N_ACT + 1  # first low-field threshold
# free-dim chunks: packed fields must stay <= 4095 counts
CHUNKS = [(0, 2731), (2731, 5462), (5462, 8192)]
NCH = len(CHUNKS)
PACK = 4096.0

F32 = mybir.dt.float32
ALU = mybir.AluOpType
ACTF = mybir.ActivationFunctionType


def _register_pack2():
    """Custom DVE op: accum = sum((x >= s0) + 4096*(x >= s1)) over free dim."""
    name = "COUNT_GE_PACK2_ANT"
    for op in dve_ops.OPS:
        if op.name == name:
            return op
    body = (Src0 >= C0) + (Src0 >= C1) * C2

    def _ref(in0, in1, s0, s1, imm2):
        b = ((in0 >= s0).astype(np.float32)
             + (in0 >= s1).astype(np.float32) * imm2)
        return b, b.reshape(b.shape[0], -1).sum(axis=-1, keepdims=True)

    spec = Spec(body=body, accum=_op_add, reference=_ref)
    row = dve_ops._CUSTOM_DVE_ROW_BASE + len(dve_ops.OPS)
    dve_ops._SUB_OPCODE_FOR_NAME[name] = row
    shas = {}
    for ver in ("v3", "v4"):
        tmp = DveOpSpec(name=name, opcode=row, uops=lower(spec, ver=ver),
                        rd1_en=False)
        shas[ver] = tmp.sha(ver)
    op = DveOp(name, spec, subdim=False, uops_sha=shas)
    dve_ops.OPS.append(op)
    dve_ops.CUSTOM_DVE_SPECS[name] = spec
    return op


PACK2 = _register_pack2()


def _build():
    nc = bacc.Bacc("TRN2", target_bir_lowering=False, debug=False,
                   num_devices=N_CORES)
    img = nc.dram_tensor("img", [SLABS, P, F], F32, kind="ExternalInput")
    params = nc.dram_tensor("params", [N_PARAMS], F32, kind="ExternalInput")
    out = nc.dram_tensor("out", [IMG_PER_CORE, OUT], F32, kind="ExternalOutput")

    img_ap = img.ap()
    par_ap = params.ap()
    out_ap = out.ap()

    with tile.TileContext(nc) as tc:
        with (
            tc.tile_pool(name="data", bufs=2) as data_pool,
            tc.tile_pool(name="work", bufs=1) as work,
            tc.tile_pool(name="psum", bufs=1, space="PSUM") as psum,
        ):
            scratch = work.tile([P, F], F32, tag="scratch")
            scratch2 = work.tile([P, F], F32, tag="scratch2")
            cnt = work.tile([P, SLABS * BINS], F32, tag="cnt")
            dcnt = work.tile([P, SLABS * BINS], F32, tag="dcnt")
            ones = work.tile([P, 1], F32, tag="ones")
            nc.vector.memset(ones[:], 1.0)
            act_biases = work.tile([P, N_ACT + 1], F32, tag="act_biases")
            for b in range(1, N_ACT + 1):
                nc.gpsimd.memset(act_biases[:, b:b + 1], -float(b) * (2.0 ** 32))
            # C_0 (count >= 0) is all of each partition's elements.
            nc.vector.memset(cnt[:, 0:SLABS * BINS:BINS], float(F))

            # --- threshold counting ---
            for s in range(SLABS):
                xt = data_pool.tile([P, F], F32, tag="xt")
                nc.sync.dma_start(out=xt[:], in_=img_ap[s])
                c0 = s * BINS
                # ACT channel: thresholds 1..N_ACT
                for b in range(1, N_ACT + 1):
                    nc.scalar.activation(
                        out=scratch2[:], in_=xt[:],
                        func=ACTF.Sigmoid,
                        scale=float(2.0 ** 38),
                        bias=act_biases[:, b:b + 1],
                        accum_out=cnt[:, c0 + b:c0 + b + 1],
                    )
                # DVE channel: packed pairs (lo=PAIR_LO+i, hi=PAIR_LO+NPAIR+i)
                acc = data_pool.tile([P, NCH * NPAIR], F32, tag="acc")
                for ch, (f0, f1) in enumerate(CHUNKS):
                    for i in range(NPAIR):
                        t_lo = float(PAIR_LO + i) / BINS
                        t_hi = float(PAIR_LO + NPAIR + i) / BINS
                        col = ch * NPAIR + i
                        nc.vector._custom_dve(
                            PACK2, out=scratch[:, f0:f1], in0=xt[:, f0:f1],
                            s0=t_lo, s1=t_hi, imm2=PACK,
                            accum_out=acc[:, col:col + 1],
                        )
                # decode packed fields: acc = clo + 4096*chi, exact fp32 ints
                W = NCH * NPAIR
                m = data_pool.tile([P, W], F32, tag="dec_m")
                r = data_pool.tile([P, W], F32, tag="dec_r")
                err = data_pool.tile([P, W], F32, tag="dec_err")
                neg = data_pool.tile([P, W], F32, tag="dec_neg")
                chi = data_pool.tile([P, W], F32, tag="dec_chi")
                clo = data_pool.tile([P, W], F32, tag="dec_clo")
                nc.vector.tensor_scalar(
                    out=m[:], in0=acc[:], scalar1=1.0 / PACK, scalar2=float(2 ** 23),
                    op0=ALU.mult, op1=ALU.add)
                nc.vector.tensor_scalar(
                    out=r[:], in0=m[:], scalar1=float(2 ** 23), scalar2=None,
                    op0=ALU.subtract)
                # err = acc - PACK*r
                nc.vector.scalar_tensor_tensor(
                    out=err[:], in0=r[:], scalar=-PACK, in1=acc[:],
                    op0=ALU.mult, op1=ALU.add)
                nc.vector.tensor_scalar(
                    out=neg[:], in0=err[:], scalar1=0.0, scalar2=None,
                    op0=ALU.is_lt)
                nc.vector.tensor_tensor(
                    out=chi[:], in0=r[:], in1=neg[:], op=ALU.subtract)
                nc.vector.scalar_tensor_tensor(
                    out=clo[:], in0=neg[:], scalar=PACK, in1=err[:],
                    op0=ALU.mult, op1=ALU.add)
                # sum the 3 chunks
                nc.vector.tensor_tensor(
                    out=clo[:, 0:NPAIR], in0=clo[:, 0:NPAIR],
                    in1=clo[:, NPAIR:2 * NPAIR], op=ALU.add)
                nc.vector.tensor_tensor(
                    out=clo[:, 0:NPAIR], in0=clo[:, 0:NPAIR],
                    in1=clo[:, 2 * NPAIR:3 * NPAIR], op=ALU.add)
                nc.vector.tensor_tensor(
                    out=chi[:, 0:NPAIR], in0=chi[:, 0:NPAIR],
                    in1=chi[:, NPAIR:2 * NPAIR], op=ALU.add)
                nc.vector.tensor_tensor(
                    out=chi[:, 0:NPAIR], in0=chi[:, 0:NPAIR],
                    in1=chi[:, 2 * NPAIR:3 * NPAIR], op=ALU.add)
                # place into cnt: low fields -> PAIR_LO.., high -> PAIR_LO+NPAIR..
                nc.vector.tensor_copy(
                    out=cnt[:, c0 + PAIR_LO:c0 + PAIR_LO + NPAIR],
                    in_=clo[:, 0:NPAIR])
                nc.vector.tensor_copy(
                    out=cnt[:, c0 + PAIR_LO + NPAIR:c0 + BINS],
                    in_=chi[:, 0:NPAIR])

            # --- hist[b] = C[b] - C[b+1]; per-partition, then reduce via PE ---
            nbm1 = SLABS * BINS - 1
            nc.vector.tensor_tensor(
                out=dcnt[:, 0:nbm1], in0=cnt[:, 0:nbm1], in1=cnt[:, 1:nbm1 + 1],
                op=ALU.subtract,
            )
            # last bin of each slab: hist[63] = C[63] (count >= 1.0 is 0)
            nc.vector.tensor_copy(
                out=dcnt[:, BINS - 1:SLABS * BINS:BINS],
                in_=cnt[:, BINS - 1:SLABS * BINS:BINS],
            )

            # featT[f, i] = hist of image i, feature f = c*64+b.
            # dcnt columns for image i are contiguous: [i*192, (i+1)*192).
            featT_a = psum.tile([P, IMG_PER_CORE], F32, tag="featTa")
            featT_b = psum.tile([BINS, IMG_PER_CORE], F32, tag="featTb")
            for i in range(IMG_PER_CORE):
                base = i * FEAT
                nc.tensor.matmul(
                    featT_a[:, i:i + 1], dcnt[:, base:base + P], ones[:],
                    start=True, stop=True,
                )
                nc.tensor.matmul(
                    featT_b[:, i:i + 1], dcnt[:, base + P:base + FEAT], ones[:],
                    start=True, stop=True,
                )

            feat_a = work.tile([P, IMG_PER_CORE], F32, tag="feata")
            feat_b = work.tile([BINS, IMG_PER_CORE], F32, tag="featb")
            nc.vector.tensor_copy(out=feat_a[:], in_=featT_a[:])
            nc.vector.tensor_copy(out=feat_b[:], in_=featT_b[:])

            # --- MLP weights from params ---
            w1a = work.tile([P, HID], F32, tag="w1a")
            w1b = work.tile([FEAT - P, HID], F32, tag="w1b")
            nc.sync.dma_start(
                out=w1a[:], in_=par_ap[0:P * HID].rearrange("(a b) -> a b", a=P))
            nc.sync.dma_start(
                out=w1b[:],
                in_=par_ap[P * HID:W1_N].rearrange("(a b) -> a b", a=FEAT - P))
            b1 = work.tile([HID, 1], F32, tag="b1")
            nc.sync.dma_start(
                out=b1[:], in_=par_ap[W1_N:W1_N + B1_N].rearrange("(a b) -> a b", a=HID))
            w2 = work.tile([HID, OUT], F32, tag="w2")
            nc.sync.dma_start(
                out=w2[:],
                in_=par_ap[W1_N + B1_N:W1_N + B1_N + W2_N].rearrange(
                    "(a b) -> a b", a=HID))
            b2 = work.tile([OUT, 1], F32, tag="b2")
            nc.sync.dma_start(
                out=b2[:],
                in_=par_ap[W1_N + B1_N + W2_N:G_OFF].rearrange("(a b) -> a b", a=OUT))
            gsc = work.tile([1, 1], F32, tag="gsc")
            nc.sync.dma_start(
                out=gsc[:], in_=par_ap[G_OFF:G_OFF + 1].rearrange("(a b) -> a b", a=1))
            ones_out = work.tile([1, OUT], F32, tag="ones_out")
            nc.vector.memset(ones_out[:], 1.0)

            # broadcast global scalar to 32 partitions via PE
            g_psum = psum.tile([OUT, 1], F32, tag="gpsum")
            nc.tensor.matmul(g_psum[:], ones_out[:], gsc[:], start=True, stop=True)
            bias2 = work.tile([OUT, 1], F32, tag="bias2")
            nc.vector.tensor_add(out=bias2[:], in0=b2[:], in1=g_psum[:])

            # --- layer 1: h = relu(featT.T @ w1 + b1), computed transposed ---
            h_psum = psum.tile([HID, IMG_PER_CORE], F32, tag="hpsum")
            nc.tensor.matmul(h_psum[:], w1a[:], feat_a[:], start=True, stop=False)
            nc.tensor.matmul(h_psum[:], w1b[:], feat_b[:], start=False, stop=True)
            h = work.tile([HID, IMG_PER_CORE], F32, tag="h")
            nc.scalar.activation(
                out=h[:], in_=h_psum[:], func=ACTF.Relu, bias=b1[:], scale=1.0)

            # --- layer 2: o = sigmoid(h.T @ w2 + b2 + g), transposed ---
            o_psum = psum.tile([OUT, IMG_PER_CORE], F32, tag="opsum")
            nc.tensor.matmul(o_psum[:], w2[:], h[:], start=True, stop=True)
            o = work.tile([OUT, IMG_PER_CORE], F32, tag="o")
            nc.scalar.activation(
                out=o[:], in_=o_psum[:], func=ACTF.Sigmoid, bias=bias2[:], scale=1.0)

            # --- store transposed [OUT, IMG] -> dram [IMG, OUT] ---
            nc.sync.dma_start(out=out_ap.rearrange("a b -> b a"), in_=o[:])

    nc.compile()
    return nc


_NC_CACHE = {}


def _get_nc():
    if "nc" not in _NC_CACHE:
        _NC_CACHE["nc"] = _build()
    return _NC_CACHE["nc"]


def kernel(img: np.ndarray, params: np.ndarray) -> np.ndarray:
    img = np.ascontiguousarray(img, dtype=np.float32)
    params = np.ascontiguousarray(params, dtype=np.float32)
    assert img.shape == (N_IMG, CH, 1024, 1024)
    assert params.shape == (N_PARAMS,)

    nc = _get_nc()
    shards = img.reshape(N_CORES, SLABS, P, F)
    in_maps = [
        {"img": shards[c], "params": params} for c in range(N_CORES)
    ]
    res = bass_utils.run_bass_kernel_spmd(nc, in_maps, core_ids=list(range(N_CORES)))
    return np.concatenate([res.results[c]["out"] for c in range(N_CORES)], axis=0)
